# revision 5
# baseline (speedup 1.0000x reference)
"""3-layer GCN (GCNConv x3 + log_softmax) on 8 Trainium2 NeuronCores — v2.

Strategy (dst-sharded graph parallel, one-hot segment-sum on TensorE):
  - Nodes partitioned into 8 ranges (12500/core). Core k owns dst range k.
  - GCN norm is folded out of the edge loop: xw rows are pre-scaled by
    dinv[src] in the GEMM epilogue (per-partition activation scale), and the
    aggregated psum is scaled by dinv[dst] (tensor_tensor with a broadcast
    dinv tile) before bias+relu. The edge matmul operand S is then a PURE
    one-hot matrix built with a single batched is_equal per 16 columns
    (broadcast-AP tensor_tensor against a tiled iota).
  - Per layer: GEMM runs per 128-node block in 4 node quarters; each quarter's
    [Q,128] bf16 product is AllGathered (Shared-output mesh collective) so
    aggregation of quarter q can start while quarter q+1 is still in flight.
  - Aggregation: edges sorted by dst within (segment of 512 dsts, src-quarter)
    units and packed into 128-slot columns on a fixed span grid (stride st,
    span 128). Each column is one LDWEIGHTS(gathered rows) + matmul with
    moving dim = actual span width, accumulated into the segment's PSUM bank.
  - Layer 3: feat-major psum -> z = dinv*psum + b3 (bf16), transposed via a
    PE identity matmul, then exp/ln/sub log_softmax with per-partition ops.

Feature data bf16 (f32 psum accumulation); gather indices int16 over 4 source
windows (one per quarter, <= 25600 rows).
"""

import os
import sys

for _p in ("/opt/trn_rl_repo",):
    if os.path.isdir(_p) and _p not in sys.path:
        sys.path.insert(0, _p)

import numpy as np
import ml_dtypes

import concourse.bacc as bacc
import concourse.bass as bass
import concourse.tile as tile
from concourse import mybir, library_config
from concourse.bass_utils import run_bass_kernel_spmd
from concourse._compat import cdiv

BF16 = mybir.dt.bfloat16
F32 = mybir.dt.float32
I16 = mybir.dt.int16
NP_BF16 = ml_dtypes.bfloat16

PAD_DM = 200.0  # sentinel dst-mod for padded slots (is_equal never fires)


# ----------------------------------------------------------------------------
# configuration
# ----------------------------------------------------------------------------
def full_cfg():
    return dict(N=100000, F=128, C=40, NCORES=8, BLK=128, SEGW=512, NQ=4,
                SK=16, SEGGRP=3)


def derive(cfg):
    d = dict(cfg)
    d["NPC"] = cfg["N"] // cfg["NCORES"]
    assert d["NPC"] * cfg["NCORES"] == cfg["N"]
    NPC, NQ, BLK = d["NPC"], cfg["NQ"], cfg["BLK"]
    # node quarters (block-aligned except the last)
    qfull = cdiv(NPC, NQ * BLK) * BLK
    qsz, off = [], 0
    for q in range(NQ):
        s = min(qfull, NPC - off)
        assert s > 0, "NPC too small for NQ quarters"
        qsz.append(s)
        off += s
    d["QSZ"] = qsz
    d["QOFF"] = list(np.cumsum([0] + qsz[:-1]))
    d["WSZ"] = [s * cfg["NCORES"] for s in qsz]  # gather window sizes
    assert max(d["WSZ"]) <= 32767
    d["NSEG"] = cdiv(NPC, cfg["SEGW"])
    d["NBLK"] = cdiv(NPC, BLK)
    return d


# ----------------------------------------------------------------------------
# schedule
# ----------------------------------------------------------------------------
class Sched:
    """Uniform (core-invariant) column layout.

    Columns ordered by (segment, quarter, column). Unit (s, q) has
    ncols[(s, q)] columns; column k spans dsts [b_k, min(b_k+128, segw))
    relative to the segment base, with breakpoints b_k derived from the
    aggregate (all-core) dst distribution.
    """

    def __init__(self, d):
        self.d = d
        self.bks = {}      # (s, q) -> breakpoint array [ncols]
        self.ncols = {}    # (s, q) -> n columns
        self.colstart = {} # (s, q) -> first global column
        self.width = []    # per global column: moving width (max over cores)
        self.spanstart = []  # per global column: span start within segment
        self.totcols = 0

    def seg_width(self, s):
        return min(self.d["SEGW"], self.d["NPC"] - s * self.d["SEGW"])


def _breakpoints(agg_sorted, ncols, segw):
    """Aggregate-quantile span starts with coverage constraints."""
    T = len(agg_sorted)
    if T == 0:
        bk = np.minimum(np.arange(ncols) * 128, max(segw - 1, 0))
    else:
        bk = agg_sorted[(np.arange(ncols) * T) // ncols].astype(np.int64)
    bk[0] = 0
    # coverage from the top: b_k >= segw - 128*(ncols-k)
    lo_req = segw - 128 * (ncols - np.arange(ncols))
    bk = np.maximum(bk, lo_req)
    # monotone + gap <= 128 (forward pass)
    for k in range(1, ncols):
        bk[k] = max(bk[k], bk[k - 1])
        bk[k] = min(bk[k], bk[k - 1] + 128)
    bk = np.minimum(bk, max(segw - 1, 0))
    bk[0] = 0
    return bk


def _pack_unit(drel, bk, segw):
    """Greedy descending-column packing against breakpoints bk.
    drel sorted ascending. Returns (col, None) on success, or
    (None, d_problem) with a dst needing extra column coverage."""
    n = len(drel)
    ncols = len(bk)
    col = np.empty(n, dtype=np.int64)
    if n == 0:
        return col, None
    top = n
    for k in range(ncols - 1, -1, -1):
        if top == 0:
            break
        if drel[top - 1] >= bk[k] + 128:
            return None, int(drel[top - 1])  # above remaining spans
        lo_k = np.searchsorted(drel[:top], bk[k])
        take = min(top - lo_k, 128)
        col[top - take:top] = k
        top -= take
    if top != 0:
        return None, int(drel[top - 1])
    return col, None


def prep_graph(d, edge_index):
    N, NPC, BLK, SEGW = d["N"], d["NPC"], d["BLK"], d["SEGW"]
    NSEG, NQ, NCORES = d["NSEG"], d["NQ"], d["NCORES"]
    QSZ, QOFF = d["QSZ"], d["QOFF"]

    src = np.asarray(edge_index[0], dtype=np.int64)
    dst = np.asarray(edge_index[1], dtype=np.int64)

    # degrees include self-loops; the self-loop term itself is applied by a
    # dense per-block transpose matmul in the kernel, not via edge slots.
    deg = (np.bincount(dst, minlength=N) + 1).astype(np.float64)
    dinv = (1.0 / np.sqrt(deg)).astype(np.float32)

    core = dst // NPC
    drel = dst % NPC
    seg = drel // SEGW
    dsegrel = drel % SEGW

    ps = src % NPC
    qsz_arr = np.asarray(QSZ)
    qoff_arr = np.asarray(QOFF)
    qq = np.minimum(ps // QSZ[0], NQ - 1)
    widx = (src // NPC) * qsz_arr[qq] + (ps - qoff_arr[qq])

    sched = Sched(d)

    # group edges per (core, seg, q)
    ukey = (core * NSEG + seg) * NQ + qq
    order = np.lexsort((dsegrel, ukey))
    uk_s = ukey[order]
    bounds = np.searchsorted(uk_s, np.arange(NCORES * NSEG * NQ + 1))

    # per-core slot arrays, filled as we fix the schedule
    col_of = np.empty(len(src), dtype=np.int64)   # global column (per edge)
    rank_of = np.empty(len(src), dtype=np.int64)  # slot within column

    # pass 1: per-unit packing
    units = {}
    for s in range(NSEG):
        segw = sched.seg_width(s)
        for q in range(NQ):
            idxs = [
                order[bounds[(c * NSEG + s) * NQ + q]:
                      bounds[(c * NSEG + s) * NQ + q + 1]]
                for c in range(NCORES)
            ]
            drs = [np.ascontiguousarray(dsegrel[ix]) for ix in idxs]
            maxload = max(len(dr) for dr in drs)
            agg = np.sort(np.concatenate(drs))
            ncols = max(cdiv(maxload, 126), cdiv(segw, 128), 1)
            bk = _breakpoints(agg, ncols, segw)
            ok = None
            for _try in range(64):
                cols_c = []
                d_prob = None
                for c in range(NCORES):
                    colc, dp = _pack_unit(drs[c], bk, segw)
                    if colc is None:
                        d_prob = dp
                        break
                    cols_c.append(colc)
                if d_prob is None:
                    ok = (bk, len(bk), cols_c)
                    break
                # insert a column covering the problem dst
                ins = max(0, min(d_prob - 96, segw - 128 if segw > 128 else 0))
                bk = np.sort(np.append(bk, ins))
                bk[0] = 0
            assert ok is not None, f"packing failed seg={s} q={q}"
            bk, ncols, cols_c = ok
            # widths per column (max over cores)
            wk = np.ones(ncols, dtype=np.int64)
            for c in range(NCORES):
                ix = idxs[c]
                if len(ix):
                    w = dsegrel[ix] - bk[cols_c[c]] + 1
                    np.maximum.at(wk, cols_c[c], w)
            wk = np.minimum(wk, segw - bk)
            # reorder columns by width descending (S-build bucketing)
            perm = np.argsort(-wk, kind="stable")
            inv = np.empty(ncols, dtype=np.int64)
            inv[perm] = np.arange(ncols)
            units[(s, q)] = (bk[perm], wk[perm], idxs,
                             [inv[cc] for cc in cols_c])

    # pass 2: assign global columns ordered (group, q, s) so each (group, q)
    # is one contiguous gather
    SEGGRP = d["SEGGRP"]
    totcols = 0
    width_all = []
    spanstart_all = []
    for s0 in range(0, NSEG, SEGGRP):
        for q in range(NQ):
            for s in range(s0, min(s0 + SEGGRP, NSEG)):
                bk, wk, idxs, cols_c = units[(s, q)]
                ncols = len(bk)
                sched.bks[(s, q)] = bk
                sched.ncols[(s, q)] = ncols
                sched.colstart[(s, q)] = totcols
                for c in range(NCORES):
                    ix = idxs[c]
                    if len(ix):
                        colc = cols_c[c]
                        o2 = np.lexsort((dsegrel[ix], colc))
                        ixo = ix[o2]
                        colo = colc[o2]
                        first = np.ones(len(ixo), dtype=bool)
                        first[1:] = colo[1:] != colo[:-1]
                        fpos = np.where(first)[0]
                        gid = np.cumsum(first) - 1
                        rank = np.arange(len(ixo)) - fpos[gid]
                        col_of[ixo] = totcols + colo
                        rank_of[ixo] = rank
                        assert rank.max(initial=0) < 128
                width_all.append(wk)
                spanstart_all.append(bk)
                totcols += ncols
    sched.totcols = totcols
    sched.width = np.concatenate(width_all) if width_all else np.zeros(0, np.int64)
    sched.spanstart = (
        np.concatenate(spanstart_all) if spanstart_all else np.zeros(0, np.int64)
    )
    SK = d["SK"]
    sched.gwidth = [
        int(sched.width[g * SK:(g + 1) * SK].max())
        for g in range(cdiv(totcols, SK))
    ]

    TOT = totcols * 128
    slot = col_of * 128 + rank_of
    per_core = []
    for c in range(NCORES):
        m = core == c
        lidx = np.zeros(TOT, dtype=np.int16)
        dm = np.full(TOT, PAD_DM, dtype=np.float32)
        sl = slot[m]
        lidx[sl] = widx[m].astype(np.int16)
        # dst-mod relative to column span start
        dm[sl] = (dsegrel[m] - sched.spanstart[col_of[m]]).astype(np.float32)
        wrapped = lidx.reshape(-1, 16).T           # [16, TOT/16]
        idx128 = np.tile(wrapped, (8, 1))          # [128, TOT*8/16]
        dm128 = dm.reshape(-1, 128).T              # [128, TOTCOLS]
        per_core.append(
            dict(
                idx=np.ascontiguousarray(idx128),
                dm=np.ascontiguousarray(dm128.astype(NP_BF16)),
            )
        )
    return sched, per_core, dinv


# ----------------------------------------------------------------------------
# kernel builder
# ----------------------------------------------------------------------------
def build(d, sched):
    N, F, C, NPC, BLK = d["N"], d["F"], d["C"], d["NPC"], d["BLK"]
    NSEG, NQ, NCORES, SEGW = d["NSEG"], d["NQ"], d["NCORES"], d["SEGW"]
    NBLK, QSZ, QOFF, WSZ = d["NBLK"], d["QSZ"], d["QOFF"], d["WSZ"]
    SK, SEGGRP = d["SK"], d["SEGGRP"]
    TOTCOLS = sched.totcols
    MAXGRPCOLS = 0
    for s0 in range(0, NSEG, SEGGRP):
        segs = list(range(s0, min(s0 + SEGGRP, NSEG)))
        for q in range(NQ):
            MAXGRPCOLS = max(
                MAXGRPCOLS, sum(sched.ncols[(s, q)] for s in segs)
            )
    NWBLK = NCORES * sum(cdiv(QSZ[q], BLK) for q in range(NQ))

    nc = bacc.Bacc(
        "TRN2",
        target_bir_lowering=False,
        debug=False,
        num_devices=NCORES,
        num_swdge_queues=4,
    )

    xT = nc.dram_tensor("xT", [F, NPC], BF16, kind="ExternalInput")
    xTfull = nc.dram_tensor("xTfull", [F, N], BF16, kind="ExternalInput")
    dinvW_in = nc.dram_tensor("dinvW", [128, NWBLK], F32, kind="ExternalInput")
    Ws = [nc.dram_tensor(f"W{i}", [F, F], BF16, kind="ExternalInput") for i in range(3)]
    b1 = nc.dram_tensor("b1", [F, 1], F32, kind="ExternalInput")
    b2 = nc.dram_tensor("b2", [F, 1], F32, kind="ExternalInput")
    dinvP_in = nc.dram_tensor("dinvP", [128, NBLK], F32, kind="ExternalInput")
    dinvB_in = nc.dram_tensor("dinvB", [128, NPC], BF16, kind="ExternalInput")
    iota_in = nc.dram_tensor("iotaT", [128, SK * 128], BF16, kind="ExternalInput")
    i128_in = nc.dram_tensor("i128", [128, 128], BF16, kind="ExternalInput")
    iaug_in = nc.dram_tensor("iaug", [65, 64], BF16, kind="ExternalInput")
    idx_in = nc.dram_tensor("idx", [128, TOTCOLS * 8], I16, kind="ExternalInput")
    dm_in = nc.dram_tensor("dm", [128, TOTCOLS], BF16, kind="ExternalInput")
    out = nc.dram_tensor("out", [NPC, C], F32, kind="ExternalOutput")
    SEGB = SEGW // BLK  # blocks per full segment

    with tile.TileContext(nc) as tc:
        with (
            tc.tile_pool(name="const", bufs=1) as constp,
            tc.tile_pool(name="h", bufs=1) as hp,
            tc.tile_pool(name="gath", bufs=2) as gathp,
            tc.tile_pool(name="sp", bufs=3) as sp,
            tc.tile_pool(name="xw", bufs=3) as xwp,
            tc.tile_pool(name="xt", bufs=3) as xtp,
            tc.tile_pool(name="ep", bufs=3) as epp,
            tc.tile_pool(name="ep2", bufs=2) as ep2,
            tc.tile_pool(name="ps_seg", bufs=4, space="PSUM") as ps_seg,
            tc.tile_pool(name="ps_gemm", bufs=2, space="PSUM") as ps_gemm,
            tc.tile_pool(name="ps_tr", bufs=2, space="PSUM") as ps_tr,
            tc.tile_pool(name="dram", bufs=1, space="DRAM") as dramp,
        ):
            nc.gpsimd.load_library(library_config.mlp)

            # resident constants
            iota = constp.tile([128, SK * 128], BF16, tag="iota")
            nc.sync.dma_start(iota[:], iota_in[:])
            i128 = constp.tile([128, 128], BF16, tag="i128")
            nc.sync.dma_start(i128[:], i128_in[:])
            iaug = constp.tile([65, 64], BF16, tag="iaug")
            nc.sync.dma_start(iaug[:], iaug_in[:])
            wt = []
            for i in range(3):
                w = constp.tile([F, F], BF16, tag=f"w{i}")
                nc.sync.dma_start(w[:], Ws[i][:])
                wt.append(w)
            b1t = constp.tile([F, 1], F32, tag="b1")
            nc.sync.dma_start(b1t[:], b1[:])
            b2t = constp.tile([F, 1], F32, tag="b2")
            nc.sync.dma_start(b2t[:], b2[:])
            dinvP = constp.tile([128, NBLK], F32, tag="dinvP")
            nc.sync.dma_start(dinvP[:], dinvP_in[:])
            dinvW = constp.tile([128, NWBLK], F32, tag="dinvW")
            nc.sync.dma_start(dinvW[:], dinvW_in[:])
            dinvB = constp.tile([128, NPC], BF16, tag="dinvB")
            nc.sync.dma_start(dinvB[:], dinvB_in[:])
            dmt = constp.tile([128, TOTCOLS], BF16, tag="dm")
            nc.sync.dma_start(dmt[:], dm_in[:])
            idxt = constp.tile([128, TOTCOLS * 8], I16, tag="idx")
            nc.sync.dma_start(idxt[:], idx_in[:])
            zero = constp.tile([128, 128], BF16, tag="zero")
            nc.vector.memset(zero[:], 0.0)
            ztaug = constp.tile([65, SEGW], BF16, tag="ztaug")
            nc.vector.memset(ztaug[:], 0.0)
            nc.vector.memset(ztaug[64:65, :], 1.0)

            hA = hp.tile([F, NPC], BF16, tag="hA")
            hB = hp.tile([F, NPC], BF16, tag="hB")
            xws = hp.tile([128, NBLK, F], BF16, tag="xws")
            nc.sync.dma_start(hA[:], xT[:])

            # DRAM staging for collectives (per quarter, double-buffered by
            # layer parity so layer L+1's AllGather never races layer L's
            # gathers)
            xw_slice = [
                [
                    dramp.tile([QSZ[q], F], BF16, tag=f"xw_slice{p}_{q}",
                               name=f"xw_slice{p}_{q}")
                    for q in range(NQ)
                ]
                for p in range(2)
            ]
            xw_full = [
                [
                    nc.dram_tensor(f"xw_full{p}_{q}", [WSZ[q], F], BF16,
                                   addr_space="Shared")
                    for q in range(NQ)
                ]
                for p in range(2)
            ]

            def emit_phaseA(L, q, hsrc):
                """GEMM of quarter q for layer L's aggregation + AllGather."""
                nblk_q = cdiv(QSZ[q], BLK)
                blk0 = QOFF[q] // BLK
                for g0 in range(0, nblk_q, 4):
                    gn = min(4, nblk_q - g0)
                    ps4 = ps_gemm.tile([128, 512], F32, tag="gemm_ps")
                    nts = []
                    for j in range(gn):
                        t = g0 + j
                        n0 = QOFF[q] + t * BLK
                        nt = min(BLK, QOFF[q] + QSZ[q] - n0)
                        gblk = n0 // BLK
                        nc.tensor.matmul(
                            ps4[:nt, j * BLK:(j + 1) * BLK],
                            hsrc[:, n0:n0 + nt],
                            wt[L][:],
                            start=True,
                            stop=True,
                        )
                        nc.scalar.activation(
                            xws[:nt, gblk, :],
                            ps4[:nt, j * BLK:(j + 1) * BLK],
                            mybir.ActivationFunctionType.Copy,
                            scale=dinvP[:nt, gblk:gblk + 1],
                        )
                        nts.append(nt)
                    r0 = g0 * BLK
                    if all(nt == BLK for nt in nts):
                        nc.sync.dma_start(
                            xw_slice[L % 2][q][r0:r0 + gn * BLK, :].rearrange(
                                "(j p) f -> p j f", p=BLK
                            ),
                            xws[:, blk0 + g0:blk0 + g0 + gn, :],
                        )
                    else:
                        for j in range(gn):
                            nt = nts[j]
                            nc.sync.dma_start(
                                xw_slice[L % 2][q][
                                    r0 + j * BLK:r0 + j * BLK + nt, :
                                ],
                                xws[:nt, blk0 + g0 + j, :],
                            )
                nc.gpsimd.collective_compute(
                    "AllGather",
                    mybir.AluOpType.bypass,
                    ins=[xw_slice[L % 2][q][:].opt()],
                    outs=[xw_full[L % 2][q][:].opt()],
                    replica_groups=[list(range(NCORES))],
                )

            def emit_phaseA0_local(q):
                """Layer 0: every core computes window q's xw locally from the
                full x (no AllGather)."""
                nbq = cdiv(QSZ[q], BLK)
                wb_base = NCORES * sum(cdiv(QSZ[p], BLK) for p in range(q))
                for sec in range(NCORES):
                    node0 = sec * NPC + QOFF[q]
                    row0 = sec * QSZ[q]
                    for g0 in range(0, nbq, 4):
                        gn = min(4, nbq - g0)
                        rows = min(4 * BLK, QSZ[q] - g0 * BLK)
                        xt4 = xtp.tile([128, 4 * BLK], BF16, tag="xt4")
                        nc.sync.dma_start(
                            xt4[:, :rows],
                            xTfull[:, node0 + g0 * BLK:node0 + g0 * BLK + rows],
                        )
                        ps4 = ps_gemm.tile([128, 512], F32, tag="gemm_ps")
                        xw4 = xtp.tile([128, 4, BLK], BF16, tag="xw4")
                        nts = []
                        for j in range(gn):
                            nt = min(BLK, rows - j * BLK)
                            wb = wb_base + sec * nbq + g0 + j
                            nc.tensor.matmul(
                                ps4[:nt, j * BLK:(j + 1) * BLK],
                                xt4[:, j * BLK:j * BLK + nt],
                                wt[0][:],
                                start=True,
                                stop=True,
                            )
                            nc.scalar.activation(
                                xw4[:nt, j, :],
                                ps4[:nt, j * BLK:(j + 1) * BLK],
                                mybir.ActivationFunctionType.Copy,
                                scale=dinvW[:nt, wb:wb + 1],
                            )
                            nts.append(nt)
                        r0 = row0 + g0 * BLK
                        if all(nt == BLK for nt in nts):
                            nc.sync.dma_start(
                                xw_full[0][q][r0:r0 + gn * BLK, :].rearrange(
                                    "(j p) f -> p j f", p=BLK
                                ),
                                xw4[:, :gn, :],
                            )
                        else:
                            for j in range(gn):
                                nt = nts[j]
                                nc.sync.dma_start(
                                    xw_full[0][q][
                                        r0 + j * BLK:r0 + j * BLK + nt, :
                                    ],
                                    xw4[:nt, j, :],
                                )

            def emit_xws_own(L, hsrc):
                """Own-block xw' into xws (feeds self-loop matmuls)."""
                for g0 in range(0, NBLK, 4):
                    gn = min(4, NBLK - g0)
                    ps4 = ps_gemm.tile([128, 512], F32, tag="gemm_ps")
                    for j in range(gn):
                        t = g0 + j
                        n0 = t * BLK
                        nt = min(BLK, NPC - n0)
                        nc.tensor.matmul(
                            ps4[:nt, j * BLK:(j + 1) * BLK],
                            hsrc[:, n0:n0 + nt],
                            wt[L][:],
                            start=True,
                            stop=True,
                        )
                        nc.scalar.activation(
                            xws[:nt, t, :],
                            ps4[:nt, j * BLK:(j + 1) * BLK],
                            mybir.ActivationFunctionType.Copy,
                            scale=dinvP[:nt, t:t + 1],
                        )

            # last segment needed before layer L+1's quarter-q GEMM can run
            qseg_end = [(QOFF[q] + QSZ[q] - 1) // SEGW for q in range(NQ)]

            hcur = hA
            gq = [0]
            for q in range(NQ):
                emit_phaseA0_local(q)
            emit_xws_own(0, hA)
            for L in range(3):
                # ---- phase B: aggregation, segment groups with q-major inner
                sgrp = {}  # global S-group id -> (tile, gw)
                hnext = hB if hcur is hA else hA
                next_q = 0  # next quarter of layer L+1 to emit phase A for
                for s0 in range(0, NSEG, SEGGRP):
                    segs = list(range(s0, min(s0 + SEGGRP, NSEG)))
                    pss = {}
                    for s in segs:
                        segw = sched.seg_width(s)
                        pssb = ps_seg.tile([128, SEGW], F32, tag="pssb")
                        pss[s] = pssb
                        nc.tensor.matmul(
                            pssb[:, :segw], zero[:], iota[:, :segw],
                            start=True, stop=False,
                        )
                        # self-loop contribution: psum[:, j] += xw'[node]^T
                        for j in range(cdiv(segw, BLK)):
                            t = s * SEGB + j
                            nt = min(BLK, segw - j * BLK)
                            nc.tensor.matmul(
                                pssb[:, j * BLK:j * BLK + nt],
                                xws[:nt, t, :],
                                i128[:nt, :nt],
                                start=False,
                                stop=False,
                            )
                    for q in range(NQ):
                        gc0 = sched.colstart[(segs[0], q)]
                        gncols = sum(sched.ncols[(s, q)] for s in segs)
                        g = gathp.tile([128, MAXGRPCOLS, F], BF16, tag="g")
                        nc.gpsimd.dma_gather(
                            g[:, :gncols, :],
                            xw_full[L % 2][q][:],
                            idxt[:, gc0 * 8:(gc0 + gncols) * 8],
                            gncols * 128,
                            gncols * 128,
                            F,
                            single_packet=False,
                            queue_num=gq[0] % 4,
                        )
                        gq[0] += 1
                        for s in segs:
                            pssb = pss[s]
                            c0 = sched.colstart[(s, q)]
                            ncols = sched.ncols[(s, q)]
                            for k in range(ncols):
                                gc = c0 + k
                                grp, gi = divmod(gc, SK)
                                if grp not in sgrp:
                                    gk = min(SK, TOTCOLS - grp * SK)
                                    gw = sched.gwidth[grp]
                                    stile = sp.tile([128, SK * 128], BF16, tag="s")
                                    sview = stile[:, :gk * gw].rearrange(
                                        "p (k w) -> p k w", w=gw
                                    )
                                    nc.vector.tensor_tensor(
                                        sview,
                                        iota[:, :gw].unsqueeze(1).broadcast_to(
                                            [128, gk, gw]
                                        ),
                                        dmt[:, grp * SK:grp * SK + gk]
                                        .unsqueeze(-1)
                                        .broadcast_to([128, gk, gw]),
                                        mybir.AluOpType.is_equal,
                                    )
                                    sgrp[grp] = (stile, gw)
                                w = int(sched.width[gc])
                                b = int(sched.spanstart[gc])
                                stile, gw = sgrp[grp]
                                nc.tensor.matmul(
                                    pssb[:, b:b + w],
                                    g[:, gc - gc0, :],
                                    stile[:, gi * gw:gi * gw + w],
                                    start=False,
                                    stop=False,
                                )
                    for s in segs:
                        pssb = pss[s]
                        segw = sched.seg_width(s)
                        nc.tensor.matmul(
                            pssb[:, :segw], zero[:], iota[:, :segw],
                            start=False, stop=True,
                        )
                        # ---- epilogue
                        n0 = s * SEGW
                        if L < 2:
                            tmp = epp.tile([128, SEGW], F32, tag="tmp")
                            nc.vector.tensor_tensor(
                                tmp[:, :segw], pssb[:, :segw],
                                dinvB[:, n0:n0 + segw], mybir.AluOpType.mult,
                            )
                            nc.scalar.activation(
                                hnext[:, n0:n0 + segw],
                                tmp[:, :segw],
                                mybir.ActivationFunctionType.Relu,
                                bias=(b1t if L == 0 else b2t)[:],
                            )
                        else:
                            nc.vector.tensor_tensor(
                                ztaug[:40, :segw], pssb[:40, :segw],
                                dinvB[:40, n0:n0 + segw], mybir.AluOpType.mult,
                            )
                            nsub = cdiv(segw, BLK)
                            pst = ps_tr.tile([128, 4 * 64], F32, tag="pst")
                            tt = ep2.tile([128, 4, 64], F32, tag="tt")
                            ss = ep2.tile([128, 4], F32, tag="ss")
                            ee = ep2.tile([128, 4, 40], F32, tag="ee")
                            lns = ep2.tile([128, 4], F32, tag="lns")
                            of = ep2.tile([128, 4, 40], F32, tag="of")
                            for j in range(nsub):
                                nt = min(BLK, segw - j * BLK)
                                nc.tensor.matmul(
                                    pst[:nt, j * 64:(j + 1) * 64],
                                    ztaug[:, j * BLK:j * BLK + nt],
                                    iaug[:],
                                    start=True,
                                    stop=True,
                                )
                                nc.scalar.activation(
                                    tt[:nt, j, :], pst[:nt, j * 64:(j + 1) * 64],
                                    mybir.ActivationFunctionType.Copy,
                                )
                                nc.scalar.activation(
                                    ee[:nt, j, :], tt[:nt, j, :40],
                                    mybir.ActivationFunctionType.Exp,
                                    accum_out=ss[:nt, j:j + 1],
                                )
                                nc.scalar.activation(
                                    lns[:nt, j:j + 1], ss[:nt, j:j + 1],
                                    mybir.ActivationFunctionType.Ln,
                                )
                                nc.vector.tensor_tensor(
                                    of[:nt, j, :], tt[:nt, j, :40],
                                    lns[:nt, j:j + 1].broadcast_to([nt, 40]),
                                    mybir.AluOpType.subtract,
                                )
                                nc.sync.dma_start(
                                    out[n0 + j * BLK:n0 + j * BLK + nt, :],
                                    of[:nt, j, :],
                                )
                    # emit next layer's GEMM+AllGather as soon as its input
                    # node range is complete
                    if L < 2:
                        done_seg = segs[-1]
                        while next_q < NQ and qseg_end[next_q] <= done_seg:
                            emit_phaseA(L + 1, next_q, hnext)
                            next_q += 1
                if L < 2:
                    hcur = hB if hcur is hA else hA

    nc.compile()
    return nc


# ----------------------------------------------------------------------------
# host-side input prep
# ----------------------------------------------------------------------------
def make_in_maps(d, sched, per_core, dinv, x, W1, b1, W2, b2, W3, b3):
    N, F, C, NPC, NCORES, NBLK, BLK, SK = (
        d["N"], d["F"], d["C"], d["NPC"], d["NCORES"], d["NBLK"], d["BLK"], d["SK"]
    )
    QSZ, QOFF, NQ = d["QSZ"], d["QOFF"], d["NQ"]
    x = np.asarray(x, dtype=np.float32)
    W3p = np.zeros((F, F), dtype=np.float32)
    W3p[:, : np.asarray(W3).shape[1]] = np.asarray(W3, dtype=np.float32)
    iaug = np.zeros((65, 64), dtype=np.float32)
    iaug[:64, :64] = np.eye(64)
    iaug[64, :C] = np.asarray(b3, dtype=np.float32)
    iota = np.tile(np.arange(128, dtype=np.float32), (128, SK))
    xTfull = np.ascontiguousarray(x.T).astype(NP_BF16)
    # dinv per window block (window-row order, quarters concatenated)
    wbcols = []
    for q in range(NQ):
        nbq = cdiv(QSZ[q], BLK)
        for sec in range(NCORES):
            base = sec * NPC + QOFF[q]
            for t in range(nbq):
                col = np.ones(128, dtype=np.float32)
                nt = min(BLK, QSZ[q] - t * BLK)
                col[:nt] = dinv[base + t * BLK:base + t * BLK + nt]
                wbcols.append(col)
    dinvW = np.stack(wbcols, axis=1)
    in_maps = []
    for c in range(NCORES):
        sl = slice(c * NPC, (c + 1) * NPC)
        dv = dinv[sl]
        dinvP = np.ones((128, NBLK), dtype=np.float32)
        for t in range(NBLK):
            nt = min(BLK, NPC - t * BLK)
            dinvP[:nt, t] = dv[t * BLK:t * BLK + nt]
        dinvB = np.broadcast_to(dv[None, :], (128, NPC))
        in_maps.append(
            {
                "xT": np.ascontiguousarray(x[sl].T).astype(NP_BF16),
                "xTfull": xTfull,
                "dinvW": dinvW,
                "W0": np.asarray(W1, dtype=np.float32).astype(NP_BF16),
                "W1": np.asarray(W2, dtype=np.float32).astype(NP_BF16),
                "W2": W3p.astype(NP_BF16),
                "b1": np.asarray(b1, dtype=np.float32).reshape(F, 1),
                "b2": np.asarray(b2, dtype=np.float32).reshape(F, 1),
                "dinvP": dinvP,
                "dinvB": np.ascontiguousarray(dinvB.astype(NP_BF16)),
                "iotaT": iota.astype(NP_BF16),
                "i128": np.eye(128, dtype=np.float32).astype(NP_BF16),
                "iaug": iaug.astype(NP_BF16),
                "idx": per_core[c]["idx"],
                "dm": per_core[c]["dm"],
            }
        )
    return in_maps


_CACHE = {}


def run(d, edge_index, x, W1, b1, W2, b2, W3, b3, trace=False, trace_kwargs=None):
    key = "nc"
    if key not in _CACHE:
        sched, per_core, dinv = prep_graph(d, edge_index)
        nc = build(d, sched)
        _CACHE[key] = (nc, sched, per_core, dinv)
    nc, sched, per_core, dinv = _CACHE[key]
    in_maps = make_in_maps(d, sched, per_core, dinv, x, W1, b1, W2, b2, W3, b3)
    res = run_bass_kernel_spmd(
        nc,
        in_maps,
        core_ids=list(range(d["NCORES"])),
        trace=trace,
        **(trace_kwargs or {}),
    )
    outs = [res.results[c]["out"] for c in range(d["NCORES"])]
    full = np.concatenate(outs, axis=0).astype(np.float32)
    return full, res


def kernel(x, edge_index, W1, b1, W2, b2, W3, b3):
    d = derive(full_cfg())
    out, _ = run(d, edge_index, x, W1, b1, W2, b2, W3, b3)
    return out


# revision 6
# speedup vs baseline: 1.4386x; 1.4386x over previous
"""3-layer GCN (GCNConv x3 + log_softmax) on 8 Trainium2 NeuronCores — v2.

Strategy (dst-sharded graph parallel, one-hot segment-sum on TensorE):
  - Nodes partitioned into 8 ranges (12500/core). Core k owns dst range k.
  - GCN norm is folded out of the edge loop: xw rows are pre-scaled by
    dinv[src] in the GEMM epilogue (per-partition activation scale), and the
    aggregated psum is scaled by dinv[dst] (tensor_tensor with a broadcast
    dinv tile) before bias+relu. The edge matmul operand S is then a PURE
    one-hot matrix built with a single batched is_equal per 16 columns
    (broadcast-AP tensor_tensor against a tiled iota).
  - Per layer: GEMM runs per 128-node block in 4 node quarters; each quarter's
    [Q,128] bf16 product is AllGathered (Shared-output mesh collective) so
    aggregation of quarter q can start while quarter q+1 is still in flight.
  - Aggregation: edges sorted by dst within (segment of 512 dsts, src-quarter)
    units and packed into 128-slot columns on a fixed span grid (stride st,
    span 128). Each column is one LDWEIGHTS(gathered rows) + matmul with
    moving dim = actual span width, accumulated into the segment's PSUM bank.
  - Layer 3: feat-major psum -> z = dinv*psum + b3 (bf16), transposed via a
    PE identity matmul, then exp/ln/sub log_softmax with per-partition ops.

Feature data bf16 (f32 psum accumulation); gather indices int16 over 4 source
windows (one per quarter, <= 25600 rows).
"""

import os
import sys

for _p in ("/opt/trn_rl_repo",):
    if os.path.isdir(_p) and _p not in sys.path:
        sys.path.insert(0, _p)

import numpy as np
import ml_dtypes

import concourse.bacc as bacc
import concourse.bass as bass
import concourse.tile as tile
from concourse import mybir, library_config
from concourse.bass_utils import run_bass_kernel_spmd
from concourse._compat import cdiv

BF16 = mybir.dt.bfloat16
F32 = mybir.dt.float32
I16 = mybir.dt.int16
NP_BF16 = ml_dtypes.bfloat16

PAD_DM = 200.0  # sentinel dst-mod for padded slots (is_equal never fires)


# ----------------------------------------------------------------------------
# configuration
# ----------------------------------------------------------------------------
def full_cfg():
    return dict(N=100000, F=128, C=40, NCORES=8, BLK=128, SEGW=512, NQ=4,
                SK=16, SEGGRP=3, GCHUNK=24)


def derive(cfg):
    d = dict(cfg)
    d["NPC"] = cfg["N"] // cfg["NCORES"]
    assert d["NPC"] * cfg["NCORES"] == cfg["N"]
    NPC, NQ, BLK = d["NPC"], cfg["NQ"], cfg["BLK"]
    # node quarters (block-aligned except the last)
    qfull = cdiv(NPC, NQ * BLK) * BLK
    qsz, off = [], 0
    for q in range(NQ):
        s = min(qfull, NPC - off)
        assert s > 0, "NPC too small for NQ quarters"
        qsz.append(s)
        off += s
    d["QSZ"] = qsz
    d["QOFF"] = list(np.cumsum([0] + qsz[:-1]))
    d["WSZ"] = [s * cfg["NCORES"] for s in qsz]  # gather window sizes
    assert max(d["WSZ"]) <= 32767
    d["NSEG"] = cdiv(NPC, cfg["SEGW"])
    d["NBLK"] = cdiv(NPC, BLK)
    return d


# ----------------------------------------------------------------------------
# schedule
# ----------------------------------------------------------------------------
class Sched:
    """Uniform (core-invariant) column layout.

    Columns ordered by (segment, quarter, column). Unit (s, q) has
    ncols[(s, q)] columns; column k spans dsts [b_k, min(b_k+128, segw))
    relative to the segment base, with breakpoints b_k derived from the
    aggregate (all-core) dst distribution.
    """

    def __init__(self, d):
        self.d = d
        self.bks = {}      # (s, q) -> breakpoint array [ncols]
        self.ncols = {}    # (s, q) -> n columns
        self.colstart = {} # (s, q) -> first global column
        self.width = []    # per global column: moving width (max over cores)
        self.spanstart = []  # per global column: span start within segment
        self.totcols = 0

    def seg_width(self, s):
        return min(self.d["SEGW"], self.d["NPC"] - s * self.d["SEGW"])


def _breakpoints(agg_sorted, ncols, segw):
    """Aggregate-quantile span starts with coverage constraints."""
    T = len(agg_sorted)
    if T == 0:
        bk = np.minimum(np.arange(ncols) * 128, max(segw - 1, 0))
    else:
        bk = agg_sorted[(np.arange(ncols) * T) // ncols].astype(np.int64)
    bk[0] = 0
    # coverage from the top: b_k >= segw - 128*(ncols-k)
    lo_req = segw - 128 * (ncols - np.arange(ncols))
    bk = np.maximum(bk, lo_req)
    # monotone + gap <= 128 (forward pass)
    for k in range(1, ncols):
        bk[k] = max(bk[k], bk[k - 1])
        bk[k] = min(bk[k], bk[k - 1] + 128)
    bk = np.minimum(bk, max(segw - 1, 0))
    bk[0] = 0
    return bk


def _pack_unit(drel, bk, segw):
    """Greedy descending-column packing against breakpoints bk.
    drel sorted ascending. Returns (col, None) on success, or
    (None, d_problem) with a dst needing extra column coverage."""
    n = len(drel)
    ncols = len(bk)
    col = np.empty(n, dtype=np.int64)
    if n == 0:
        return col, None
    top = n
    for k in range(ncols - 1, -1, -1):
        if top == 0:
            break
        if drel[top - 1] >= bk[k] + 128:
            return None, int(drel[top - 1])  # above remaining spans
        lo_k = np.searchsorted(drel[:top], bk[k])
        take = min(top - lo_k, 128)
        col[top - take:top] = k
        top -= take
    if top != 0:
        return None, int(drel[top - 1])
    return col, None


def prep_graph(d, edge_index):
    N, NPC, BLK, SEGW = d["N"], d["NPC"], d["BLK"], d["SEGW"]
    NSEG, NQ, NCORES = d["NSEG"], d["NQ"], d["NCORES"]
    QSZ, QOFF = d["QSZ"], d["QOFF"]

    src = np.asarray(edge_index[0], dtype=np.int64)
    dst = np.asarray(edge_index[1], dtype=np.int64)

    # degrees include self-loops; the self-loop term itself is applied by a
    # dense per-block transpose matmul in the kernel, not via edge slots.
    deg = (np.bincount(dst, minlength=N) + 1).astype(np.float64)
    dinv = (1.0 / np.sqrt(deg)).astype(np.float32)

    core = dst // NPC
    drel = dst % NPC
    seg = drel // SEGW
    dsegrel = drel % SEGW

    ps = src % NPC
    qsz_arr = np.asarray(QSZ)
    qoff_arr = np.asarray(QOFF)
    qq = np.minimum(ps // QSZ[0], NQ - 1)
    widx = (src // NPC) * qsz_arr[qq] + (ps - qoff_arr[qq])

    sched = Sched(d)

    # group edges per (core, seg, q)
    ukey = (core * NSEG + seg) * NQ + qq
    order = np.lexsort((dsegrel, ukey))
    uk_s = ukey[order]
    bounds = np.searchsorted(uk_s, np.arange(NCORES * NSEG * NQ + 1))

    # per-core slot arrays, filled as we fix the schedule
    col_of = np.empty(len(src), dtype=np.int64)   # global column (per edge)
    rank_of = np.empty(len(src), dtype=np.int64)  # slot within column

    # pass 1: per-unit packing
    units = {}
    for s in range(NSEG):
        segw = sched.seg_width(s)
        for q in range(NQ):
            idxs = [
                order[bounds[(c * NSEG + s) * NQ + q]:
                      bounds[(c * NSEG + s) * NQ + q + 1]]
                for c in range(NCORES)
            ]
            drs = [np.ascontiguousarray(dsegrel[ix]) for ix in idxs]
            maxload = max(len(dr) for dr in drs)
            agg = np.sort(np.concatenate(drs))
            ncols = max(cdiv(maxload, 126), cdiv(segw, 128), 1)
            bk = _breakpoints(agg, ncols, segw)
            ok = None
            for _try in range(64):
                cols_c = []
                d_prob = None
                for c in range(NCORES):
                    colc, dp = _pack_unit(drs[c], bk, segw)
                    if colc is None:
                        d_prob = dp
                        break
                    cols_c.append(colc)
                if d_prob is None:
                    ok = (bk, len(bk), cols_c)
                    break
                # insert a column covering the problem dst
                ins = max(0, min(d_prob - 96, segw - 128 if segw > 128 else 0))
                bk = np.sort(np.append(bk, ins))
                bk[0] = 0
            assert ok is not None, f"packing failed seg={s} q={q}"
            bk, ncols, cols_c = ok
            # widths per column (max over cores)
            wk = np.ones(ncols, dtype=np.int64)
            for c in range(NCORES):
                ix = idxs[c]
                if len(ix):
                    w = dsegrel[ix] - bk[cols_c[c]] + 1
                    np.maximum.at(wk, cols_c[c], w)
            wk = np.minimum(wk, segw - bk)
            # reorder columns by width descending (S-build bucketing)
            perm = np.argsort(-wk, kind="stable")
            inv = np.empty(ncols, dtype=np.int64)
            inv[perm] = np.arange(ncols)
            units[(s, q)] = (bk[perm], wk[perm], idxs,
                             [inv[cc] for cc in cols_c])

    # pass 2: assign global columns ordered (group, q, s) so each (group, q)
    # is one contiguous gather
    SEGGRP = d["SEGGRP"]
    totcols = 0
    width_all = []
    spanstart_all = []
    for s0 in range(0, NSEG, SEGGRP):
        for q in range(NQ):
            for s in range(s0, min(s0 + SEGGRP, NSEG)):
                bk, wk, idxs, cols_c = units[(s, q)]
                ncols = len(bk)
                sched.bks[(s, q)] = bk
                sched.ncols[(s, q)] = ncols
                sched.colstart[(s, q)] = totcols
                for c in range(NCORES):
                    ix = idxs[c]
                    if len(ix):
                        colc = cols_c[c]
                        o2 = np.lexsort((dsegrel[ix], colc))
                        ixo = ix[o2]
                        colo = colc[o2]
                        first = np.ones(len(ixo), dtype=bool)
                        first[1:] = colo[1:] != colo[:-1]
                        fpos = np.where(first)[0]
                        gid = np.cumsum(first) - 1
                        rank = np.arange(len(ixo)) - fpos[gid]
                        col_of[ixo] = totcols + colo
                        rank_of[ixo] = rank
                        assert rank.max(initial=0) < 128
                width_all.append(wk)
                spanstart_all.append(bk)
                totcols += ncols
    sched.totcols = totcols
    sched.width = np.concatenate(width_all) if width_all else np.zeros(0, np.int64)
    sched.spanstart = (
        np.concatenate(spanstart_all) if spanstart_all else np.zeros(0, np.int64)
    )
    SK = d["SK"]
    sched.gwidth = [
        int(sched.width[g * SK:(g + 1) * SK].max())
        for g in range(cdiv(totcols, SK))
    ]

    TOT = totcols * 128
    slot = col_of * 128 + rank_of
    per_core = []
    for c in range(NCORES):
        m = core == c
        lidx = np.zeros(TOT, dtype=np.int16)
        dm = np.full(TOT, PAD_DM, dtype=np.float32)
        sl = slot[m]
        lidx[sl] = widx[m].astype(np.int16)
        # dst-mod relative to column span start
        dm[sl] = (dsegrel[m] - sched.spanstart[col_of[m]]).astype(np.float32)
        wrapped = lidx.reshape(-1, 16).T           # [16, TOT/16]
        idx128 = np.tile(wrapped, (8, 1))          # [128, TOT*8/16]
        dm128 = dm.reshape(-1, 128).T              # [128, TOTCOLS]
        per_core.append(
            dict(
                idx=np.ascontiguousarray(idx128),
                dm=np.ascontiguousarray(dm128.astype(NP_BF16)),
            )
        )
    return sched, per_core, dinv


# ----------------------------------------------------------------------------
# kernel builder
# ----------------------------------------------------------------------------
def build(d, sched):
    N, F, C, NPC, BLK = d["N"], d["F"], d["C"], d["NPC"], d["BLK"]
    NSEG, NQ, NCORES, SEGW = d["NSEG"], d["NQ"], d["NCORES"], d["SEGW"]
    NBLK, QSZ, QOFF, WSZ = d["NBLK"], d["QSZ"], d["QOFF"], d["WSZ"]
    SK, SEGGRP, GCHUNK = d["SK"], d["SEGGRP"], d["GCHUNK"]
    TOTCOLS = sched.totcols
    MAXGRPCOLS = 0
    for s0 in range(0, NSEG, SEGGRP):
        segs = list(range(s0, min(s0 + SEGGRP, NSEG)))
        for q in range(NQ):
            MAXGRPCOLS = max(
                MAXGRPCOLS, sum(sched.ncols[(s, q)] for s in segs)
            )
    NWBLK = NCORES * sum(cdiv(QSZ[q], BLK) for q in range(NQ))

    nc = bacc.Bacc(
        "TRN2",
        target_bir_lowering=False,
        debug=False,
        num_devices=NCORES,
        num_swdge_queues=4,
    )

    xT = nc.dram_tensor("xT", [F, NPC], BF16, kind="ExternalInput")
    xTfull = nc.dram_tensor("xTfull", [F, N], BF16, kind="ExternalInput")
    dinvW_in = nc.dram_tensor("dinvW", [128, NWBLK], F32, kind="ExternalInput")
    Ws = [nc.dram_tensor(f"W{i}", [F, F], BF16, kind="ExternalInput") for i in range(3)]
    b1 = nc.dram_tensor("b1", [F, 1], F32, kind="ExternalInput")
    b2 = nc.dram_tensor("b2", [F, 1], F32, kind="ExternalInput")
    dinvP_in = nc.dram_tensor("dinvP", [128, NBLK], F32, kind="ExternalInput")
    dinvB_in = nc.dram_tensor("dinvB", [128, NPC], BF16, kind="ExternalInput")
    iota_in = nc.dram_tensor("iotaT", [128, SK * 128], BF16, kind="ExternalInput")
    i128_in = nc.dram_tensor("i128", [128, 128], BF16, kind="ExternalInput")
    iaug_in = nc.dram_tensor("iaug", [65, 64], BF16, kind="ExternalInput")
    idx_in = nc.dram_tensor("idx", [128, TOTCOLS * 8], I16, kind="ExternalInput")
    dm_in = nc.dram_tensor("dm", [128, TOTCOLS], BF16, kind="ExternalInput")
    out = nc.dram_tensor("out", [NPC, C], F32, kind="ExternalOutput")
    SEGB = SEGW // BLK  # blocks per full segment

    with tile.TileContext(nc) as tc:
        with (
            tc.tile_pool(name="const", bufs=1) as constp,
            tc.tile_pool(name="h", bufs=1) as hp,
            tc.tile_pool(name="gath", bufs=2) as gathp,
            tc.tile_pool(name="sp", bufs=3) as sp,
            tc.tile_pool(name="xw", bufs=3) as xwp,
            tc.tile_pool(name="xt", bufs=2) as xtp,
            tc.tile_pool(name="ep", bufs=3) as epp,
            tc.tile_pool(name="ep2", bufs=2) as ep2,
            tc.tile_pool(name="ps_seg", bufs=4, space="PSUM") as ps_seg,
            tc.tile_pool(name="ps_gemm", bufs=2, space="PSUM") as ps_gemm,
            tc.tile_pool(name="ps_tr", bufs=2, space="PSUM") as ps_tr,
            tc.tile_pool(name="dram", bufs=1, space="DRAM") as dramp,
        ):
            nc.gpsimd.load_library(library_config.mlp)

            # resident constants
            iota = constp.tile([128, SK * 128], BF16, tag="iota")
            nc.sync.dma_start(iota[:], iota_in[:])
            i128 = constp.tile([128, 128], BF16, tag="i128")
            nc.sync.dma_start(i128[:], i128_in[:])
            iaug = constp.tile([65, 64], BF16, tag="iaug")
            nc.sync.dma_start(iaug[:], iaug_in[:])
            wt = []
            for i in range(3):
                w = constp.tile([F, F], BF16, tag=f"w{i}")
                nc.sync.dma_start(w[:], Ws[i][:])
                wt.append(w)
            b1t = constp.tile([F, 1], F32, tag="b1")
            nc.sync.dma_start(b1t[:], b1[:])
            b2t = constp.tile([F, 1], F32, tag="b2")
            nc.sync.dma_start(b2t[:], b2[:])
            dinvP = constp.tile([128, NBLK], F32, tag="dinvP")
            nc.sync.dma_start(dinvP[:], dinvP_in[:])
            dinvW = constp.tile([128, NWBLK], F32, tag="dinvW")
            nc.sync.dma_start(dinvW[:], dinvW_in[:])
            dinvB = constp.tile([128, NPC], BF16, tag="dinvB")
            nc.sync.dma_start(dinvB[:], dinvB_in[:])
            dmt = constp.tile([128, TOTCOLS], BF16, tag="dm")
            nc.sync.dma_start(dmt[:], dm_in[:])
            idxt = constp.tile([128, TOTCOLS * 8], I16, tag="idx")
            nc.sync.dma_start(idxt[:], idx_in[:])
            zero = constp.tile([128, 128], BF16, tag="zero")
            nc.vector.memset(zero[:], 0.0)
            ztaug = constp.tile([65, SEGW], BF16, tag="ztaug")
            nc.vector.memset(ztaug[:], 0.0)
            nc.vector.memset(ztaug[64:65, :], 1.0)

            hA = hp.tile([F, NPC], BF16, tag="hA")
            hB = hp.tile([F, NPC], BF16, tag="hB")
            xws = hp.tile([128, NBLK, F], BF16, tag="xws")
            nc.sync.dma_start(hA[:], xT[:])

            # DRAM staging for collectives (per quarter, double-buffered by
            # layer parity so layer L+1's AllGather never races layer L's
            # gathers)
            xw_slice = [
                [
                    dramp.tile([QSZ[q], F], BF16, tag=f"xw_slice{p}_{q}",
                               name=f"xw_slice{p}_{q}")
                    for q in range(NQ)
                ]
                for p in range(2)
            ]
            xw_full = [
                [
                    nc.dram_tensor(f"xw_full{p}_{q}", [WSZ[q], F], BF16,
                                   addr_space="Shared")
                    for q in range(NQ)
                ]
                for p in range(2)
            ]

            def emit_phaseA(L, q, hsrc):
                """GEMM of quarter q for layer L's aggregation + AllGather."""
                nblk_q = cdiv(QSZ[q], BLK)
                blk0 = QOFF[q] // BLK
                for g0 in range(0, nblk_q, 4):
                    gn = min(4, nblk_q - g0)
                    ps4 = ps_gemm.tile([128, 512], F32, tag="gemm_ps")
                    nts = []
                    for j in range(gn):
                        t = g0 + j
                        n0 = QOFF[q] + t * BLK
                        nt = min(BLK, QOFF[q] + QSZ[q] - n0)
                        gblk = n0 // BLK
                        nc.tensor.matmul(
                            ps4[:nt, j * BLK:(j + 1) * BLK],
                            hsrc[:, n0:n0 + nt],
                            wt[L][:],
                            start=True,
                            stop=True,
                        )
                        nc.scalar.activation(
                            xws[:nt, gblk, :],
                            ps4[:nt, j * BLK:(j + 1) * BLK],
                            mybir.ActivationFunctionType.Copy,
                            scale=dinvP[:nt, gblk:gblk + 1],
                        )
                        nts.append(nt)
                    r0 = g0 * BLK
                    if all(nt == BLK for nt in nts):
                        nc.sync.dma_start(
                            xw_slice[L % 2][q][r0:r0 + gn * BLK, :].rearrange(
                                "(j p) f -> p j f", p=BLK
                            ),
                            xws[:, blk0 + g0:blk0 + g0 + gn, :],
                        )
                    else:
                        for j in range(gn):
                            nt = nts[j]
                            nc.sync.dma_start(
                                xw_slice[L % 2][q][
                                    r0 + j * BLK:r0 + j * BLK + nt, :
                                ],
                                xws[:nt, blk0 + g0 + j, :],
                            )
                nc.gpsimd.collective_compute(
                    "AllGather",
                    mybir.AluOpType.bypass,
                    ins=[xw_slice[L % 2][q][:].opt()],
                    outs=[xw_full[L % 2][q][:].opt()],
                    replica_groups=[list(range(NCORES))],
                )

            def emit_phaseA0_local(q):
                """Layer 0: every core computes window q's xw locally from the
                full x (no AllGather). 16-block IO chunks."""
                CHB = 16
                nbq = cdiv(QSZ[q], BLK)
                wb_base = NCORES * sum(cdiv(QSZ[p], BLK) for p in range(q))
                for sec in range(NCORES):
                    node0 = sec * NPC + QOFF[q]
                    row0 = sec * QSZ[q]
                    for c0b in range(0, nbq, CHB):
                        cn = min(CHB, nbq - c0b)
                        rows = min(CHB * BLK, QSZ[q] - c0b * BLK)
                        xt16 = xtp.tile([128, CHB * BLK], BF16, tag="xt16")
                        nc.sync.dma_start(
                            xt16[:, :rows],
                            xTfull[:, node0 + c0b * BLK:
                                   node0 + c0b * BLK + rows],
                        )
                        xw16 = xtp.tile([128, CHB, BLK], BF16, tag="xw16")
                        nts = []
                        for j4 in range(0, cn, 4):
                            ps4 = ps_gemm.tile([128, 512], F32, tag="gemm_ps")
                            for j in range(j4, min(j4 + 4, cn)):
                                nt = min(BLK, rows - j * BLK)
                                wb = wb_base + sec * nbq + c0b + j
                                nc.tensor.matmul(
                                    ps4[:nt, (j % 4) * BLK:(j % 4 + 1) * BLK],
                                    xt16[:, j * BLK:j * BLK + nt],
                                    wt[0][:],
                                    start=True,
                                    stop=True,
                                )
                                nc.scalar.activation(
                                    xw16[:nt, j, :],
                                    ps4[:nt, (j % 4) * BLK:(j % 4 + 1) * BLK],
                                    mybir.ActivationFunctionType.Copy,
                                    scale=dinvW[:nt, wb:wb + 1],
                                )
                                nts.append(nt)
                        r0 = row0 + c0b * BLK
                        if all(nt == BLK for nt in nts):
                            nc.sync.dma_start(
                                xw_full[0][q][r0:r0 + cn * BLK, :].rearrange(
                                    "(j p) f -> p j f", p=BLK
                                ),
                                xw16[:, :cn, :],
                            )
                        else:
                            for j in range(cn):
                                nt = nts[j]
                                nc.sync.dma_start(
                                    xw_full[0][q][
                                        r0 + j * BLK:r0 + j * BLK + nt, :
                                    ],
                                    xw16[:nt, j, :],
                                )

            def emit_xws_own(L, hsrc):
                """Own-block xw' into xws (feeds self-loop matmuls)."""
                for g0 in range(0, NBLK, 4):
                    gn = min(4, NBLK - g0)
                    ps4 = ps_gemm.tile([128, 512], F32, tag="gemm_ps")
                    for j in range(gn):
                        t = g0 + j
                        n0 = t * BLK
                        nt = min(BLK, NPC - n0)
                        nc.tensor.matmul(
                            ps4[:nt, j * BLK:(j + 1) * BLK],
                            hsrc[:, n0:n0 + nt],
                            wt[L][:],
                            start=True,
                            stop=True,
                        )
                        nc.scalar.activation(
                            xws[:nt, t, :],
                            ps4[:nt, j * BLK:(j + 1) * BLK],
                            mybir.ActivationFunctionType.Copy,
                            scale=dinvP[:nt, t:t + 1],
                        )

            # last segment needed before layer L+1's quarter-q GEMM can run
            qseg_end = [(QOFF[q] + QSZ[q] - 1) // SEGW for q in range(NQ)]

            hcur = hA
            gq = [0]
            for q in range(NQ):
                emit_phaseA0_local(q)
            emit_xws_own(0, hA)
            for L in range(3):
                # ---- phase B: aggregation, segment groups with q-major inner
                sgrp = {}  # global S-group id -> (tile, gw)
                hnext = hB if hcur is hA else hA
                next_q = 0  # next quarter of layer L+1 to emit phase A for
                for s0 in range(0, NSEG, SEGGRP):
                    segs = list(range(s0, min(s0 + SEGGRP, NSEG)))
                    pss = {}
                    for s in segs:
                        segw = sched.seg_width(s)
                        pssb = ps_seg.tile([128, SEGW], F32, tag="pssb")
                        pss[s] = pssb
                        nc.tensor.matmul(
                            pssb[:, :segw], zero[:], iota[:, :segw],
                            start=True, stop=False,
                        )
                        # self-loop contribution: psum[:, j] += xw'[node]^T
                        for j in range(cdiv(segw, BLK)):
                            t = s * SEGB + j
                            nt = min(BLK, segw - j * BLK)
                            nc.tensor.matmul(
                                pssb[:, j * BLK:j * BLK + nt],
                                xws[:nt, t, :],
                                i128[:nt, :nt],
                                start=False,
                                stop=False,
                            )
                    for q in range(NQ):
                        gc0 = sched.colstart[(segs[0], q)]
                        gncols = sum(sched.ncols[(s, q)] for s in segs)
                        g = gathp.tile([128, MAXGRPCOLS, F], BF16, tag="g")
                        for cc in range(0, gncols, GCHUNK):
                            ncw = min(GCHUNK, gncols - cc)
                            nc.gpsimd.dma_gather(
                                g[:, cc:cc + ncw, :],
                                xw_full[L % 2][q][:],
                                idxt[:, (gc0 + cc) * 8:(gc0 + cc + ncw) * 8],
                                ncw * 128,
                                ncw * 128,
                                F,
                                single_packet=False,
                                queue_num=gq[0] % 4,
                            )
                            gq[0] += 1
                        for s in segs:
                            pssb = pss[s]
                            c0 = sched.colstart[(s, q)]
                            ncols = sched.ncols[(s, q)]
                            for k in range(ncols):
                                gc = c0 + k
                                grp, gi = divmod(gc, SK)
                                if grp not in sgrp:
                                    gk = min(SK, TOTCOLS - grp * SK)
                                    gw = sched.gwidth[grp]
                                    stile = sp.tile([128, SK * 128], BF16, tag="s")
                                    sview = stile[:, :gk * gw].rearrange(
                                        "p (k w) -> p k w", w=gw
                                    )
                                    nc.vector.tensor_tensor(
                                        sview,
                                        iota[:, :gw].unsqueeze(1).broadcast_to(
                                            [128, gk, gw]
                                        ),
                                        dmt[:, grp * SK:grp * SK + gk]
                                        .unsqueeze(-1)
                                        .broadcast_to([128, gk, gw]),
                                        mybir.AluOpType.is_equal,
                                    )
                                    sgrp[grp] = (stile, gw)
                                w = int(sched.width[gc])
                                b = int(sched.spanstart[gc])
                                stile, gw = sgrp[grp]
                                nc.tensor.matmul(
                                    pssb[:, b:b + w],
                                    g[:, gc - gc0, :],
                                    stile[:, gi * gw:gi * gw + w],
                                    start=False,
                                    stop=False,
                                )
                    for s in segs:
                        pssb = pss[s]
                        segw = sched.seg_width(s)
                        nc.tensor.matmul(
                            pssb[:, :segw], zero[:], iota[:, :segw],
                            start=False, stop=True,
                        )
                        # ---- epilogue
                        n0 = s * SEGW
                        if L < 2:
                            tmp = epp.tile([128, SEGW], F32, tag="tmp")
                            nc.vector.tensor_tensor(
                                tmp[:, :segw], pssb[:, :segw],
                                dinvB[:, n0:n0 + segw], mybir.AluOpType.mult,
                            )
                            nc.scalar.activation(
                                hnext[:, n0:n0 + segw],
                                tmp[:, :segw],
                                mybir.ActivationFunctionType.Relu,
                                bias=(b1t if L == 0 else b2t)[:],
                            )
                        else:
                            nc.vector.tensor_tensor(
                                ztaug[:40, :segw], pssb[:40, :segw],
                                dinvB[:40, n0:n0 + segw], mybir.AluOpType.mult,
                            )
                            nsub = cdiv(segw, BLK)
                            pst = ps_tr.tile([128, 4 * 64], F32, tag="pst")
                            tt = ep2.tile([128, 4, 64], F32, tag="tt")
                            ss = ep2.tile([128, 4], F32, tag="ss")
                            ee = ep2.tile([128, 4, 40], F32, tag="ee")
                            lns = ep2.tile([128, 4], F32, tag="lns")
                            of = ep2.tile([128, 4, 40], F32, tag="of")
                            for j in range(nsub):
                                nt = min(BLK, segw - j * BLK)
                                nc.tensor.matmul(
                                    pst[:nt, j * 64:(j + 1) * 64],
                                    ztaug[:, j * BLK:j * BLK + nt],
                                    iaug[:],
                                    start=True,
                                    stop=True,
                                )
                                nc.scalar.activation(
                                    tt[:nt, j, :], pst[:nt, j * 64:(j + 1) * 64],
                                    mybir.ActivationFunctionType.Copy,
                                )
                                nc.scalar.activation(
                                    ee[:nt, j, :], tt[:nt, j, :40],
                                    mybir.ActivationFunctionType.Exp,
                                    accum_out=ss[:nt, j:j + 1],
                                )
                                nc.scalar.activation(
                                    lns[:nt, j:j + 1], ss[:nt, j:j + 1],
                                    mybir.ActivationFunctionType.Ln,
                                )
                                nc.vector.tensor_tensor(
                                    of[:nt, j, :], tt[:nt, j, :40],
                                    lns[:nt, j:j + 1].broadcast_to([nt, 40]),
                                    mybir.AluOpType.subtract,
                                )
                                nc.sync.dma_start(
                                    out[n0 + j * BLK:n0 + j * BLK + nt, :],
                                    of[:nt, j, :],
                                )
                    # emit next layer's GEMM+AllGather as soon as its input
                    # node range is complete
                    if L < 2:
                        done_seg = segs[-1]
                        while next_q < NQ and qseg_end[next_q] <= done_seg:
                            emit_phaseA(L + 1, next_q, hnext)
                            next_q += 1
                if L < 2:
                    hcur = hB if hcur is hA else hA

    nc.compile()
    return nc


# ----------------------------------------------------------------------------
# host-side input prep
# ----------------------------------------------------------------------------
def make_in_maps(d, sched, per_core, dinv, x, W1, b1, W2, b2, W3, b3):
    N, F, C, NPC, NCORES, NBLK, BLK, SK = (
        d["N"], d["F"], d["C"], d["NPC"], d["NCORES"], d["NBLK"], d["BLK"], d["SK"]
    )
    QSZ, QOFF, NQ = d["QSZ"], d["QOFF"], d["NQ"]
    x = np.asarray(x, dtype=np.float32)
    W3p = np.zeros((F, F), dtype=np.float32)
    W3p[:, : np.asarray(W3).shape[1]] = np.asarray(W3, dtype=np.float32)
    iaug = np.zeros((65, 64), dtype=np.float32)
    iaug[:64, :64] = np.eye(64)
    iaug[64, :C] = np.asarray(b3, dtype=np.float32)
    iota = np.tile(np.arange(128, dtype=np.float32), (128, SK))
    xTfull = np.ascontiguousarray(x.T).astype(NP_BF16)
    # dinv per window block (window-row order, quarters concatenated)
    wbcols = []
    for q in range(NQ):
        nbq = cdiv(QSZ[q], BLK)
        for sec in range(NCORES):
            base = sec * NPC + QOFF[q]
            for t in range(nbq):
                col = np.ones(128, dtype=np.float32)
                nt = min(BLK, QSZ[q] - t * BLK)
                col[:nt] = dinv[base + t * BLK:base + t * BLK + nt]
                wbcols.append(col)
    dinvW = np.stack(wbcols, axis=1)
    in_maps = []
    for c in range(NCORES):
        sl = slice(c * NPC, (c + 1) * NPC)
        dv = dinv[sl]
        dinvP = np.ones((128, NBLK), dtype=np.float32)
        for t in range(NBLK):
            nt = min(BLK, NPC - t * BLK)
            dinvP[:nt, t] = dv[t * BLK:t * BLK + nt]
        dinvB = np.broadcast_to(dv[None, :], (128, NPC))
        in_maps.append(
            {
                "xT": np.ascontiguousarray(x[sl].T).astype(NP_BF16),
                "xTfull": xTfull,
                "dinvW": dinvW,
                "W0": np.asarray(W1, dtype=np.float32).astype(NP_BF16),
                "W1": np.asarray(W2, dtype=np.float32).astype(NP_BF16),
                "W2": W3p.astype(NP_BF16),
                "b1": np.asarray(b1, dtype=np.float32).reshape(F, 1),
                "b2": np.asarray(b2, dtype=np.float32).reshape(F, 1),
                "dinvP": dinvP,
                "dinvB": np.ascontiguousarray(dinvB.astype(NP_BF16)),
                "iotaT": iota.astype(NP_BF16),
                "i128": np.eye(128, dtype=np.float32).astype(NP_BF16),
                "iaug": iaug.astype(NP_BF16),
                "idx": per_core[c]["idx"],
                "dm": per_core[c]["dm"],
            }
        )
    return in_maps


_CACHE = {}


def run(d, edge_index, x, W1, b1, W2, b2, W3, b3, trace=False, trace_kwargs=None):
    key = "nc"
    if key not in _CACHE:
        sched, per_core, dinv = prep_graph(d, edge_index)
        nc = build(d, sched)
        _CACHE[key] = (nc, sched, per_core, dinv)
    nc, sched, per_core, dinv = _CACHE[key]
    in_maps = make_in_maps(d, sched, per_core, dinv, x, W1, b1, W2, b2, W3, b3)
    res = run_bass_kernel_spmd(
        nc,
        in_maps,
        core_ids=list(range(d["NCORES"])),
        trace=trace,
        **(trace_kwargs or {}),
    )
    outs = [res.results[c]["out"] for c in range(d["NCORES"])]
    full = np.concatenate(outs, axis=0).astype(np.float32)
    return full, res


def kernel(x, edge_index, W1, b1, W2, b2, W3, b3):
    d = derive(full_cfg())
    out, _ = run(d, edge_index, x, W1, b1, W2, b2, W3, b3)
    return out
